# revision 45
# baseline (speedup 1.0000x reference)
"""Causal self-attention (GQA + RoPE) Trainium2 Bass kernel, 8-core SPMD.

Problem shapes (hardcoded): B=2, T=2048, C=2048, NH=16, NKV=4, HD=128.

Sharding: 8 cores = (batch b in {0,1}) x (kv-group g in {0..3}).
Core c = b*4+g handles batch b, q-heads 4g..4g+3, kv-head g.
  - Wq column-parallel (512 cols/core), Wk/Wv column-parallel (128 cols/core),
    Wproj row-parallel (512 rows/core) -> per-core partial [T, C] outputs,
    host sums the 4 partials per batch.

Per-core dataflow ("transposed flash"):
  - Inputs pre-rearranged on host so every weight/x load is one big DMA
    with contiguous per-partition lines; all loads issued at t=0 spread
    over the sync/scalar/vector/gpsimd queues.
  - Projections computed in transposed layout: qT/kT [HD, T]
    (lhsT = W chunk, rhs = xT chunk), V transposed to [T, HD] via PE.
  - RoPE on qT/kT via partition-shifted SBUF copies + cos/sin tables.
  - Scores computed transposed: S^T[tk, tq] = matmul(lhsT=kT block, rhs=qT).
    No mask preload: exp runs on raw scores and the causal triangle of the
    diagonal 128-chunk is zeroed afterwards by a [128,128] vector multiply.
  - exp without max subtraction (scores are O(5) here; safe in fp32).
  - Row sums L[tq]: exp tiles are accumulated in bf16 SBUF tiles by the
    vector engine (even tk blocks) and gpsimd/pool engine (odd blocks);
    one 2-chain ones-matmul per head reduces the partition dim on PE.
  - 1/L: DMA-reshape L [1,512] -> [128,4] so the reciprocal runs on 128
    lanes (~60ns instead of 3.3us), then DRAM-bounce broadcast to
    [128,512]; normalization on DVE.
  - Phase B software-pipelined: S for block j+2 issued before PV_j so the
    PE never waits on the scalar-engine exp.
  - Wproj row-parallel partials in bf16; out-proj chunks for tq group g-1
    woven between attention blocks of group g to fill PE slack while the
    scalar engine runs exp; psum->sbuf copies alternate vector/gpsimd.
"""

import numpy as np

import concourse.bass as bass
import concourse.bacc as bacc
import concourse.mybir as mybir
import concourse.tile as tile

B, T, C = 2, 2048, 2048
NH, NKV, HD = 16, 4, 128
P = 128
W = 512            # wide tile (PSUM bank = 512 fp32)
TB = T // P        # 16 t blocks
CB = C // P        # 16 c chunks
G = T // W         # 4 tq groups
NQ = 4             # q heads per core

F32 = mybir.dt.float32

USE_F32R = False
MM = mybir.dt.bfloat16    # matmul-input compute dtype


def build_nc():
    nc = bacc.Bacc("TRN2", target_bir_lowering=False)
    xR = nc.dram_tensor("xR", (P, G * CB * W), MM, kind="ExternalInput")[:]
    wqr = nc.dram_tensor("wqr", (P, CB * NQ * HD), MM, kind="ExternalInput")[:]
    wkr = nc.dram_tensor("wkr", (P, CB * HD), MM, kind="ExternalInput")[:]
    wvr = nc.dram_tensor("wvr", (P, CB * HD), MM, kind="ExternalInput")[:]
    wpr = nc.dram_tensor("wpr", (P, NQ * C), MM, kind="ExternalInput")[:]
    cosT = nc.dram_tensor("cosT", (P, T), MM, kind="ExternalInput")[:]
    msinT = nc.dram_tensor("msinT", (P, T), MM, kind="ExternalInput")[:]
    mask01 = nc.dram_tensor("mask01", (P, P), MM, kind="ExternalInput")[:]
    ident = nc.dram_tensor("ident", (P, P), MM, kind="ExternalInput")[:]
    onescol = nc.dram_tensor("onescol", (P, 1), MM, kind="ExternalInput")[:]
    onesrow = nc.dram_tensor("onesrow", (1, P), F32, kind="ExternalInput")[:]
    out = nc.dram_tensor("out", (T, C), MM, kind="ExternalOutput")[:]

    EXP = mybir.ActivationFunctionType.Exp

    xr4 = xR.rearrange("p (t cb w) -> p t cb w", t=G, cb=CB)
    wq3 = wqr.rearrange("p (cb m) -> p cb m", cb=CB)
    wk3 = wkr.rearrange("p (cb m) -> p cb m", cb=CB)
    wv3 = wvr.rearrange("p (cb m) -> p cb m", cb=CB)
    wp3 = wpr.rearrange("p (hb c) -> p hb c", hb=NQ)

    with tile.TileContext(nc) as tc:
        with (
            tc.tile_pool(name="singles", bufs=1) as singles,
            tc.tile_pool(name="stage", bufs=3) as stage,
            tc.tile_pool(name="ptp", bufs=4) as ptp,
            tc.tile_pool(name="accp", bufs=2) as accp,
            tc.tile_pool(name="outp", bufs=2) as outp,
            tc.tile_pool(name="small", bufs=2) as small,
            tc.tile_pool(name="dramp", bufs=4, space="DRAM") as dramp,
        ):
            # ---- resident tiles ----
            qT = singles.tile([P, NQ, T], MM)       # roped q
            yT = singles.tile([P, NQ, T], MM)       # attention out (pre-proj)
            kT = singles.tile([P, T], MM)           # roped k, [hd, t]
            Vt = singles.tile([P, TB, HD], MM)      # [t_in_blk, blk, hd]
            cos_s = singles.tile([P, T], MM)
            msin_s = singles.tile([P, T], MM)
            mask_s = singles.tile([P, P], MM)       # 0/1 causal triangle
            id_s = singles.tile([P, P], MM)
            ones_s = singles.tile([P, 1], MM)       # column of ones
            onesr_s = singles.tile([1, P], F32)     # row of ones (outer lhsT)
            wqp = [singles.tile([P, 4, NQ * HD], MM, name=f"wqp{i}")
                   for i in range(4)]
            wkall = singles.tile([P, CB, HD], MM)
            wvall = singles.tile([P, CB, HD], MM)
            wpall = singles.tile([P, NQ, C], MM)

            # ---- all loads issued up front, spread over 4 queues ----
            # sync: weights + tables (first eighth of wq tiny so the very
            # first matmul chain can start ~7us in).
            nc.sync.dma_start(out=wqp[0][:, 0:1, :], in_=wq3[:, 0:1, :])
            nc.sync.dma_start(out=wqp[0][:, 1:2, :], in_=wq3[:, 1:2, :])
            nc.sync.dma_start(out=wqp[0][:, 2:4, :], in_=wq3[:, 2:4, :])
            nc.sync.dma_start(out=wqp[1], in_=wq3[:, 4:8, :])
            nc.sync.dma_start(out=wqp[2], in_=wq3[:, 8:12, :])
            nc.sync.dma_start(out=wqp[3], in_=wq3[:, 12:16, :])
            nc.sync.dma_start(out=cos_s, in_=cosT)
            nc.sync.dma_start(out=msin_s, in_=msinT)
            nc.sync.dma_start(out=wkall, in_=wk3)
            nc.sync.dma_start(out=wvall, in_=wv3)
            nc.sync.dma_start(out=mask_s, in_=mask01)
            nc.sync.dma_start(out=id_s, in_=ident)
            nc.sync.dma_start(out=ones_s, in_=onescol)
            nc.sync.dma_start(out=onesr_s, in_=onesrow)

            def rope_apply(dst, praw, tsl):
                # dst[d,:] = praw[d,:]*cos[d,:] + rot(praw)[d,:]*msin[d,:]
                # rot swaps halves; the rotate-half sign is folded into msin.
                tmp = stage.tile([P, W], F32, tag="ropetmp", bufs=2)
                nc.gpsimd.dma_start(out=tmp[0:64, :], in_=praw[64:128, :])
                nc.gpsimd.dma_start(out=tmp[64:128, :], in_=praw[0:64, :])
                nc.vector.tensor_mul(out=dst, in0=praw, in1=cos_s[:, tsl])
                nc.vector.tensor_mul(out=tmp, in0=tmp, in1=msin_s[:, tsl])
                nc.vector.tensor_add(out=dst, in0=dst, in1=tmp)

            # ---- phase A: QKV projections + rope + V transpose ----
            def lhs_a(m, c):
                if m < 4:
                    return wqp[c // 4][:, c % 4, m * P:(m + 1) * P]
                w = wkall if m == 4 else wvall
                return w[:, c, 0:P]

            with (
                tc.tile_pool(name="pa", bufs=1, space="PSUM") as pa,
                tc.tile_pool(name="xin", bufs=2) as xin,
            ):
                xts = [xin.tile([P, CB, W], MM, tag="x", bufs=4,
                                name=f"x_t{t}") for t in range(G)]
                # x group 0 per-chunk so the first chain streams at DMA
                # arrival rate instead of waiting on monolithic transfers.
                for cb in range(CB):
                    nc.scalar.dma_start(out=xts[0][:, cb:cb + 1, :],
                                        in_=xr4[:, 0, cb:cb + 1, :])
                # later x groups + wp issued mid-phase-A so the early
                # DMA bandwidth all goes to wq/x0
                a_hooks = {
                    (0, 1): lambda: nc.gpsimd.dma_start(
                        out=xts[1], in_=xr4[:, 1, :, :]),
                    (0, 2): lambda: nc.scalar.dma_start(
                        out=xts[2], in_=xr4[:, 2, :, :]),
                    (0, 4): lambda: nc.gpsimd.dma_start(
                        out=xts[3], in_=xr4[:, 3, :, :]),
                    (1, 0): lambda: nc.scalar.dma_start(
                        out=wpall, in_=wp3),
                }
                for t in range(G):
                    _phase_a_group(nc, tc, pa, stage, xts[t], t,
                                   lhs_a, qT, kT, Vt, id_s, rope_apply,
                                   a_hooks)

            # ---- phases B+C psum pools (A's pool released above) ----
            import contextlib
            ctx_bc = contextlib.ExitStack()
            pmm = ctx_bc.enter_context(
                tc.tile_pool(name="pmm", bufs=1, space="PSUM"))
            pacc = ctx_bc.enter_context(
                tc.tile_pool(name="pacc", bufs=1, space="PSUM"))
            plps = ctx_bc.enter_context(
                tc.tile_pool(name="plps", bufs=1, space="PSUM"))
            prb = ctx_bc.enter_context(
                tc.tile_pool(name="prb", bufs=1, space="PSUM"))

            def emit_S(g, h, j, gsl):
                """Score matmul for tk block j; returns (psum tile, colslice).
                No mask preload; the diagonal triangle is zeroed post-exp."""
                jj = j - g * 4
                vst = max(jj, 0) * P
                vsl = slice(vst, W)
                sps = pmm.tile([P, W], F32, tag="mm", bufs=3, name="sps")
                nc.tensor.matmul(sps[:, vsl],
                                 kT[:, j * P:(j + 1) * P],
                                 qT[:, h, g * W + vst:(g + 1) * W],
                                 start=True, stop=True)
                return sps, vsl

            ost_state = {}

            def make_c_chunks(gg, last):
                """Output projection for tq group gg as 16 (i, cc) closures."""
                work = []
                for i in range(4 * gg, 4 * gg + 4):
                    for cc in range(4):
                        def chunk(i=i, cc=cc):
                            if cc == 0:
                                ost_state[0] = outp.tile([P, C], MM,
                                                         tag="ost", name="ost")
                            ost = ost_state[0]
                            csl = slice(cc * W, (cc + 1) * W)
                            ops = pmm.tile([P, W], F32, tag="mm", bufs=3,
                                           name="ops")
                            for hb in range(NQ):
                                nc.tensor.matmul(
                                    ops,
                                    yT[:, hb, i * P:(i + 1) * P],
                                    wpall[:, hb, csl],
                                    start=(hb == 0), stop=(hb == NQ - 1))
                            if cc % 4 == 3:
                                nc.vector.tensor_copy(out=ost[:, csl],
                                                      in_=ops)
                            else:
                                nc.scalar.copy(out=ost[:, csl], in_=ops)
                            if last:
                                # final group: store per-cc to shrink tail
                                q = (nc.sync, nc.gpsimd, nc.sync,
                                     nc.scalar)[cc]
                                q.dma_start(out=out[i * P:(i + 1) * P, csl],
                                            in_=ost[:, csl])
                            elif cc == 3:
                                q = nc.sync if i % 2 == 0 else nc.gpsimd
                                q.dma_start(out=out[i * P:(i + 1) * P, :],
                                            in_=ost)
                        work.append(chunk)
                return work

            # ---- phase B: attention, with phase-C chunks woven in ----
            # normalization: L via two N=512 ones-matmuls; 1/L via a
            # bit-trick seed + one Newton iteration on gpsimd (SBUF-only,
            # off every critical engine); broadcast over partitions with
            # a single K=1 outer product on PE.  Stage 1 (Newton) fires
            # one head later, stage 2 (broadcast+mul) at that head's last
            # block, so cross-engine latency never stalls anyone.
            MAGIC = 0x7EF311C3
            norm_s1, norm_s2 = [], []

            def norm_stage1(lsb):
                r0 = small.tile([1, W], F32, tag="nr0", bufs=1, name="nr0")
                nc.vector.tensor_scalar(
                    out=r0.bitcast(mybir.dt.uint32),
                    in0=lsb.bitcast(mybir.dt.uint32),
                    scalar1=0xFFFFFFFF, scalar2=None,
                    op0=mybir.AluOpType.bitwise_xor)
                # MAGIC - bits == (~bits) - (~MAGIC): avoids u32 wraparound
                nc.vector.tensor_scalar(
                    out=r0.bitcast(mybir.dt.uint32),
                    in0=r0.bitcast(mybir.dt.uint32),
                    scalar1=0xFFFFFFFF - MAGIC, scalar2=None,
                    op0=mybir.AluOpType.subtract)
                t = small.tile([1, W], F32, tag="nt", bufs=1, name="nt")
                nc.gpsimd.tensor_mul(out=t, in0=lsb, in1=r0)
                nc.gpsimd.tensor_scalar(
                    out=t, in0=t, scalar1=-1.0, scalar2=2.0,
                    op0=mybir.AluOpType.mult, op1=mybir.AluOpType.add)
                rcp = small.tile([1, W], F32, tag="nrcp", name="nrcp")
                nc.gpsimd.tensor_mul(out=rcp, in0=t, in1=r0)
                return rcp

            def norm_stage2(ysb, rcp, h, gsl):
                rb = prb.tile([P, W], F32, tag="rb", bufs=1, name="rb")
                nc.tensor.matmul(rb, onesr_s, rcp, start=True, stop=True)
                nc.vector.tensor_mul(out=yT[:, h, gsl], in0=ysb, in1=rb)

            for g in range(G):
                gsl = slice(g * W, (g + 1) * W)
                nblk = 4 * (g + 1)   # causal: tk blocks 0..4(g+1)-1
                cwork = make_c_chunks(g - 1, last=False) if g > 0 else []
                stride = max(1, (2 * nblk) // len(cwork)) if cwork else 0
                ci = 0
                bc = 0
                for h in range(NQ):
                    yps = pacc.tile([P, W], F32, tag="acc", bufs=2,
                                    name="yps")
                    # two exp-sum accumulators: acc_a owned by DVE
                    # (j%4 != 3), acc_b owned by gpsimd (j%4 == 3) -- no
                    # cross-engine read-modify-write on either tile.
                    acc_a = accp.tile([P, W], MM, tag="acca", name="acca")
                    acc_b = accp.tile([P, W], MM, tag="accb", name="accb")
                    if nblk < 5:
                        # g=0: acc_b's only block (j=3) covers cols 384:512
                        nc.gpsimd.memset(acc_b[:, 0:3 * P], 0.0)
                    first_b = True
                    pend = [emit_S(g, h, 0, gsl)]
                    if nblk > 1:
                        pend.append(emit_S(g, h, 1, gsl))
                    for j in range(nblk):
                        if j + 2 < nblk:
                            pend.append(emit_S(g, h, j + 2, gsl))
                        sps, vsl = pend[j]
                        pt = ptp.tile([P, W], MM, tag="pt", bufs=6, name="pt")
                        nc.scalar.activation(out=pt[:, vsl], in_=sps[:, vsl],
                                             func=EXP)
                        jj = j - g * 4
                        if jj >= 0:
                            dsl = slice(jj * P, (jj + 1) * P)
                            nc.vector.tensor_mul(out=pt[:, dsl],
                                                 in0=pt[:, dsl], in1=mask_s)
                        nc.tensor.matmul(yps[:, vsl], Vt[:, j, :], pt[:, vsl],
                                         start=(j == 0), stop=(j == nblk - 1))
                        if j % 4 == 3:
                            if first_b:
                                # full-width copies are slow on gpsimd
                                nc.vector.tensor_copy(out=acc_b[:, vsl],
                                                      in_=pt[:, vsl])
                                first_b = False
                            else:
                                nc.gpsimd.tensor_add(out=acc_b[:, vsl],
                                                     in0=acc_b[:, vsl],
                                                     in1=pt[:, vsl])
                        elif j == 0:
                            nc.vector.tensor_copy(out=acc_a, in_=pt)
                        else:
                            nc.vector.tensor_add(out=acc_a[:, vsl],
                                                 in0=acc_a[:, vsl],
                                                 in1=pt[:, vsl])
                        bc += 1
                        if j == 1:
                            if norm_s2:
                                norm_s2.pop(0)()
                            if norm_s1:
                                norm_s1.pop()()
                        if h >= 2 and cwork and ci < len(cwork) \
                                and bc % stride == 0:
                            cwork[ci]()
                            ci += 1
                    # L[tq] = colsum(acc_a + acc_b) via 2-chain ones-matmul
                    lps = plps.tile([1, W], F32, tag="lps", bufs=2,
                                    name="lps")
                    nc.tensor.matmul(lps, ones_s, acc_a,
                                     start=True, stop=False)
                    nc.tensor.matmul(lps, ones_s, acc_b,
                                     start=False, stop=True)
                    lsb = small.tile([1, W], F32, tag="lsb", name="lsb")
                    nc.vector.tensor_copy(out=lsb, in_=lps)
                    # free the psum bank now; normalize later from SBUF
                    ysb = stage.tile([P, W], F32, tag="ysb", bufs=2, name="ysb")
                    nc.scalar.copy(out=ysb, in_=yps)

                    def s1(lsb=lsb, ysb=ysb, h=h, gsl=gsl):
                        rcp = norm_stage1(lsb)
                        norm_s2.append(
                            lambda: norm_stage2(ysb, rcp, h, gsl))
                    norm_s1.append(s1)
                while ci < len(cwork):
                    cwork[ci]()
                    ci += 1
            while norm_s1:
                norm_s1.pop()()
            while norm_s2:
                norm_s2.pop(0)()
            for ch in make_c_chunks(G - 1, last=True):
                ch()
            ctx_bc.close()

    nc.compile()
    return nc


def _phase_a_group(nc, tc, pa, stage, x_t, t, lhs_a, qT, kT, Vt, id_s,
                   rope_apply, a_hooks):
    tsl = slice(t * W, (t + 1) * W)
    for m in range(6):
        hook = a_hooks.pop((t, m), None)
        if hook is not None:
            hook()
        ps = pa.tile([P, W], F32, tag="psA", bufs=3, name="ps")
        for c in range(CB):
            nc.tensor.matmul(ps, lhs_a(m, c), x_t[:, c, :],
                             start=(c == 0), stop=(c == CB - 1))
        if m < 4:
            praw = stage.tile([P, W], F32, tag="raw", bufs=4, name="praw")
            nc.scalar.copy(out=praw, in_=ps)
            rope_apply(qT[:, m, tsl], praw, tsl)
        elif m == 4:
            praw = stage.tile([P, W], F32, tag="raw", bufs=4, name="praw")
            nc.scalar.copy(out=praw, in_=ps)
            rope_apply(kT[:, tsl], praw, tsl)
        else:
            vraw = stage.tile([P, W], MM, tag="vraw", name="vraw")
            nc.vector.tensor_copy(out=vraw, in_=ps)
            for jj in range(4):
                j = t * 4 + jj
                pvt = pa.tile([P, P], MM, tag="pvt", bufs=2, name="pvt")
                nc.tensor.transpose(pvt, vraw[:, jj * P:(jj + 1) * P], id_s)
                nc.vector.tensor_copy(out=Vt[:, j, :], in_=pvt)


def make_tables():
    inv = (10000.0 ** (-(np.arange(64, dtype=np.float32) / np.float32(64.0)))
           ).astype(np.float32)
    freqs = np.arange(T, dtype=np.float32)[:, None] * inv[None, :]   # [T, 64]
    cos64 = np.cos(freqs).T.astype(np.float32)                       # [64, T]
    sin64 = np.sin(freqs).T.astype(np.float32)
    cosT = np.concatenate([cos64, cos64], axis=0)                    # [128, T]
    msinT = np.concatenate([-sin64, sin64], axis=0)
    # [P, P] 0/1 mask: mask[tk, tq] = 1 where tk <= tq (causal allowed)
    mask = np.where(
        np.arange(P)[:, None] <= np.arange(P)[None, :],
        np.float32(1.0), np.float32(0.0))
    ident = np.eye(P, dtype=np.float32)
    return cosT, msinT, mask, ident


def _rearr(a, p=P):
    """[R, M] with R = n*p -> [p, n*M] so each partition line is
    contiguous in DRAM: out[pp, n*M + m] = a[n*p + pp, m]."""
    R, M = a.shape
    n = R // p
    return np.ascontiguousarray(
        a.reshape(n, p, M).transpose(1, 0, 2).reshape(p, n * M))


def _rearr_x(xT):
    """xT [C, T] -> [P, G*CB*W], t-group major: out[p, ((t*CB)+cb)*W + w]
    = xT[cb*P + p, t*W + w], so each (p, t) line is CB*W contiguous."""
    a = xT.reshape(CB, P, G, W).transpose(1, 2, 0, 3)
    return np.ascontiguousarray(a.reshape(P, G * CB * W))


def shard_inputs(x, Wq, Wk, Wv, Wproj):
    import ml_dtypes
    bf16 = ml_dtypes.bfloat16
    cosT, msinT, mask, ident = make_tables()
    scale = np.float32(1.0 / np.sqrt(np.float32(HD)))
    xRb = [_rearr_x(np.ascontiguousarray(x[b].T)).astype(bf16)
           for b in range(B)]
    in_maps = []
    for core in range(8):
        b, g = core // 4, core % 4
        in_maps.append({
            "xR": xRb[b],
            "wqr": _rearr(Wq[:, g * NQ * HD:(g + 1) * NQ * HD] * scale
                          ).astype(bf16),
            "wkr": _rearr(Wk[:, g * HD:(g + 1) * HD]).astype(bf16),
            "wvr": _rearr(Wv[:, g * HD:(g + 1) * HD]).astype(bf16),
            "wpr": _rearr(Wproj[g * NQ * HD:(g + 1) * NQ * HD, :]).astype(bf16),
            "cosT": cosT.astype(bf16), "msinT": msinT.astype(bf16),
            "mask01": mask.astype(bf16),
            "ident": ident.astype(bf16),
            "onescol": np.ones((P, 1), dtype=bf16),
            "onesrow": np.ones((1, P), dtype=np.float32),
        })
    return in_maps


_NC_CACHE = {}


def _get_nc():
    key = USE_F32R
    if key not in _NC_CACHE:
        _NC_CACHE[key] = build_nc()
    return _NC_CACHE[key]


def kernel(x, Wq, Wk, Wv, Wproj):
    from concourse.bass_utils import run_bass_kernel_spmd
    x = np.asarray(x, dtype=np.float32)
    Wq = np.asarray(Wq, dtype=np.float32)
    Wk = np.asarray(Wk, dtype=np.float32)
    Wv = np.asarray(Wv, dtype=np.float32)
    Wproj = np.asarray(Wproj, dtype=np.float32)
    nc = _get_nc()
    in_maps = shard_inputs(x, Wq, Wk, Wv, Wproj)
    res = run_bass_kernel_spmd(nc, in_maps, core_ids=list(range(8)))
    out = np.zeros((B, T, C), dtype=np.float32)
    for core in range(8):
        b = core // 4
        out[b] += np.asarray(res.results[core]["out"], dtype=np.float32)
    return out


# revision 46
# speedup vs baseline: 1.0096x; 1.0096x over previous
"""Causal self-attention (GQA + RoPE) Trainium2 Bass kernel, 8-core SPMD.

Problem shapes (hardcoded): B=2, T=2048, C=2048, NH=16, NKV=4, HD=128.

Sharding: 8 cores = (batch b in {0,1}) x (kv-group g in {0..3}).
Core c = b*4+g handles batch b, q-heads 4g..4g+3, kv-head g.
  - Wq column-parallel (512 cols/core), Wk/Wv column-parallel (128 cols/core),
    Wproj row-parallel (512 rows/core) -> per-core partial [T, C] outputs,
    host sums the 4 partials per batch.

Per-core dataflow ("transposed flash"):
  - Inputs pre-rearranged on host so every weight/x load is one big DMA
    with contiguous per-partition lines; all loads issued at t=0 spread
    over the sync/scalar/vector/gpsimd queues.
  - Projections computed in transposed layout: qT/kT [HD, T]
    (lhsT = W chunk, rhs = xT chunk), V transposed to [T, HD] via PE.
  - RoPE on qT/kT via partition-shifted SBUF copies + cos/sin tables.
  - Scores computed transposed: S^T[tk, tq] = matmul(lhsT=kT block, rhs=qT).
    No mask preload: exp runs on raw scores and the causal triangle of the
    diagonal 128-chunk is zeroed afterwards by a [128,128] vector multiply.
  - exp without max subtraction (scores are O(5) here; safe in fp32).
  - Row sums L[tq]: exp tiles are accumulated in bf16 SBUF tiles by the
    vector engine (even tk blocks) and gpsimd/pool engine (odd blocks);
    one 2-chain ones-matmul per head reduces the partition dim on PE.
  - 1/L: DMA-reshape L [1,512] -> [128,4] so the reciprocal runs on 128
    lanes (~60ns instead of 3.3us), then DRAM-bounce broadcast to
    [128,512]; normalization on DVE.
  - Phase B software-pipelined: S for block j+2 issued before PV_j so the
    PE never waits on the scalar-engine exp.
  - Wproj row-parallel partials in bf16; out-proj chunks for tq group g-1
    woven between attention blocks of group g to fill PE slack while the
    scalar engine runs exp; psum->sbuf copies alternate vector/gpsimd.
"""

import numpy as np

import concourse.bass as bass
import concourse.bacc as bacc
import concourse.mybir as mybir
import concourse.tile as tile

B, T, C = 2, 2048, 2048
NH, NKV, HD = 16, 4, 128
P = 128
W = 512            # wide tile (PSUM bank = 512 fp32)
TB = T // P        # 16 t blocks
CB = C // P        # 16 c chunks
G = T // W         # 4 tq groups
NQ = 4             # q heads per core

F32 = mybir.dt.float32

USE_F32R = False
MM = mybir.dt.bfloat16    # matmul-input compute dtype


def build_nc():
    nc = bacc.Bacc("TRN2", target_bir_lowering=False)
    xR = nc.dram_tensor("xR", (P, G * CB * W), MM, kind="ExternalInput")[:]
    wqr = nc.dram_tensor("wqr", (P, CB * NQ * HD), MM, kind="ExternalInput")[:]
    wkr = nc.dram_tensor("wkr", (P, CB * HD), MM, kind="ExternalInput")[:]
    wvr = nc.dram_tensor("wvr", (P, CB * HD), MM, kind="ExternalInput")[:]
    wpr = nc.dram_tensor("wpr", (P, NQ * C), MM, kind="ExternalInput")[:]
    cosT = nc.dram_tensor("cosT", (P, T), MM, kind="ExternalInput")[:]
    msinT = nc.dram_tensor("msinT", (P, T), MM, kind="ExternalInput")[:]
    mask01 = nc.dram_tensor("mask01", (P, P), MM, kind="ExternalInput")[:]
    ident = nc.dram_tensor("ident", (P, P), MM, kind="ExternalInput")[:]
    onescol = nc.dram_tensor("onescol", (P, 1), MM, kind="ExternalInput")[:]
    onesrow = nc.dram_tensor("onesrow", (1, P), F32, kind="ExternalInput")[:]
    out = nc.dram_tensor("out", (T, C), MM, kind="ExternalOutput")[:]

    EXP = mybir.ActivationFunctionType.Exp

    xr4 = xR.rearrange("p (t cb w) -> p t cb w", t=G, cb=CB)
    wq3 = wqr.rearrange("p (cb m) -> p cb m", cb=CB)
    wk3 = wkr.rearrange("p (cb m) -> p cb m", cb=CB)
    wv3 = wvr.rearrange("p (cb m) -> p cb m", cb=CB)
    wp3 = wpr.rearrange("p (hb c) -> p hb c", hb=NQ)

    with tile.TileContext(nc) as tc:
        with (
            tc.tile_pool(name="singles", bufs=1) as singles,
            tc.tile_pool(name="stage", bufs=3) as stage,
            tc.tile_pool(name="ptp", bufs=4) as ptp,
            tc.tile_pool(name="accp", bufs=2) as accp,
            tc.tile_pool(name="outp", bufs=2) as outp,
            tc.tile_pool(name="small", bufs=2) as small,
            tc.tile_pool(name="dramp", bufs=4, space="DRAM") as dramp,
        ):
            # ---- resident tiles ----
            qT = singles.tile([P, NQ, T], MM)       # roped q
            yT = singles.tile([P, NQ, T], MM)       # attention out (pre-proj)
            kT = singles.tile([P, T], MM)           # roped k, [hd, t]
            Vt = singles.tile([P, TB, HD], MM)      # [t_in_blk, blk, hd]
            cos_s = singles.tile([P, T], MM)
            msin_s = singles.tile([P, T], MM)
            mask_s = singles.tile([P, P], MM)       # 0/1 causal triangle
            id_s = singles.tile([P, P], MM)
            ones_s = singles.tile([P, 1], MM)       # column of ones
            onesr_s = singles.tile([1, P], F32)     # row of ones (outer lhsT)
            wqp = [singles.tile([P, 4, NQ * HD], MM, name=f"wqp{i}")
                   for i in range(4)]
            wkall = singles.tile([P, CB, HD], MM)
            wvall = singles.tile([P, CB, HD], MM)
            wpall = singles.tile([P, NQ, C], MM)

            # ---- all loads issued up front, spread over 4 queues ----
            # sync: weights + tables (first eighth of wq tiny so the very
            # first matmul chain can start ~7us in).
            nc.sync.dma_start(out=wqp[0][:, 0:1, :], in_=wq3[:, 0:1, :])
            nc.sync.dma_start(out=wqp[0][:, 1:2, :], in_=wq3[:, 1:2, :])
            nc.sync.dma_start(out=wqp[0][:, 2:4, :], in_=wq3[:, 2:4, :])
            nc.sync.dma_start(out=wqp[1], in_=wq3[:, 4:8, :])
            nc.sync.dma_start(out=wqp[2], in_=wq3[:, 8:12, :])
            nc.sync.dma_start(out=wqp[3], in_=wq3[:, 12:16, :])
            nc.sync.dma_start(out=cos_s, in_=cosT)
            nc.sync.dma_start(out=msin_s, in_=msinT)
            nc.sync.dma_start(out=wkall, in_=wk3)
            nc.sync.dma_start(out=wvall, in_=wv3)
            nc.sync.dma_start(out=mask_s, in_=mask01)
            nc.sync.dma_start(out=id_s, in_=ident)
            nc.sync.dma_start(out=ones_s, in_=onescol)
            nc.sync.dma_start(out=onesr_s, in_=onesrow)

            def rope_apply(dst, praw, tsl):
                # dst[d,:] = praw[d,:]*cos[d,:] + rot(praw)[d,:]*msin[d,:]
                # rot swaps halves; the rotate-half sign is folded into msin.
                tmp = stage.tile([P, W], F32, tag="ropetmp", bufs=2)
                nc.gpsimd.dma_start(out=tmp[0:64, :], in_=praw[64:128, :])
                nc.gpsimd.dma_start(out=tmp[64:128, :], in_=praw[0:64, :])
                nc.vector.tensor_mul(out=dst, in0=praw, in1=cos_s[:, tsl])
                nc.vector.tensor_mul(out=tmp, in0=tmp, in1=msin_s[:, tsl])
                nc.vector.tensor_add(out=dst, in0=dst, in1=tmp)

            # ---- phase A: QKV projections + rope + V transpose ----
            def lhs_a(m, c):
                if m < 4:
                    return wqp[c // 4][:, c % 4, m * P:(m + 1) * P]
                w = wkall if m == 4 else wvall
                return w[:, c, 0:P]

            with (
                tc.tile_pool(name="pa", bufs=1, space="PSUM") as pa,
                tc.tile_pool(name="xin", bufs=2) as xin,
            ):
                xts = [xin.tile([P, CB, W], MM, tag="x", bufs=4,
                                name=f"x_t{t}") for t in range(G)]
                # x group 0 per-chunk so the first chain streams at DMA
                # arrival rate instead of waiting on monolithic transfers.
                for cb in range(CB):
                    nc.scalar.dma_start(out=xts[0][:, cb:cb + 1, :],
                                        in_=xr4[:, 0, cb:cb + 1, :])
                # later x groups + wp issued mid-phase-A so the early
                # DMA bandwidth all goes to wq/x0
                a_hooks = {
                    (0, 1): lambda: nc.gpsimd.dma_start(
                        out=xts[1], in_=xr4[:, 1, :, :]),
                    (0, 2): lambda: nc.scalar.dma_start(
                        out=xts[2], in_=xr4[:, 2, :, :]),
                    (0, 4): lambda: nc.gpsimd.dma_start(
                        out=xts[3], in_=xr4[:, 3, :, :]),
                    (1, 0): lambda: nc.scalar.dma_start(
                        out=wpall, in_=wp3),
                }
                for t in range(G):
                    _phase_a_group(nc, tc, pa, stage, xts[t], t,
                                   lhs_a, qT, kT, Vt, id_s, rope_apply,
                                   a_hooks)

            # ---- phases B+C psum pools (A's pool released above) ----
            import contextlib
            ctx_bc = contextlib.ExitStack()
            pmm = ctx_bc.enter_context(
                tc.tile_pool(name="pmm", bufs=1, space="PSUM"))
            pacc = ctx_bc.enter_context(
                tc.tile_pool(name="pacc", bufs=1, space="PSUM"))
            plps = ctx_bc.enter_context(
                tc.tile_pool(name="plps", bufs=1, space="PSUM"))
            prb = ctx_bc.enter_context(
                tc.tile_pool(name="prb", bufs=1, space="PSUM"))

            def emit_S(g, h, j, gsl):
                """Score matmul for tk block j; returns (psum tile, colslice).
                No mask preload; the diagonal triangle is zeroed post-exp."""
                jj = j - g * 4
                vst = max(jj, 0) * P
                vsl = slice(vst, W)
                sps = pmm.tile([P, W], F32, tag="mm", bufs=3, name="sps")
                nc.tensor.matmul(sps[:, vsl],
                                 kT[:, j * P:(j + 1) * P],
                                 qT[:, h, g * W + vst:(g + 1) * W],
                                 start=True, stop=True)
                return sps, vsl

            ost_state = {}

            def make_c_chunks(gg, last):
                """Output projection for tq group gg as 16 (i, cc) closures."""
                work = []
                for i in range(4 * gg, 4 * gg + 4):
                    for cc in range(4):
                        def chunk(i=i, cc=cc):
                            if cc == 0:
                                ost_state[0] = outp.tile([P, C], MM,
                                                         tag="ost", name="ost")
                            ost = ost_state[0]
                            csl = slice(cc * W, (cc + 1) * W)
                            ops = pmm.tile([P, W], F32, tag="mm", bufs=3,
                                           name="ops")
                            for hb in range(NQ):
                                nc.tensor.matmul(
                                    ops,
                                    yT[:, hb, i * P:(i + 1) * P],
                                    wpall[:, hb, csl],
                                    start=(hb == 0), stop=(hb == NQ - 1))
                            if cc % 4 == 3:
                                nc.vector.tensor_copy(out=ost[:, csl],
                                                      in_=ops)
                            else:
                                nc.scalar.copy(out=ost[:, csl], in_=ops)
                            if last:
                                # final group: store per-cc to shrink tail
                                q = (nc.sync, nc.gpsimd, nc.sync,
                                     nc.scalar)[cc]
                                q.dma_start(out=out[i * P:(i + 1) * P, csl],
                                            in_=ost[:, csl])
                            elif cc == 3:
                                q = nc.sync if i % 2 == 0 else nc.gpsimd
                                q.dma_start(out=out[i * P:(i + 1) * P, :],
                                            in_=ost)
                        work.append(chunk)
                return work

            # ---- phase B: attention, with phase-C chunks woven in ----
            # normalization: L via two N=512 ones-matmuls; 1/L via a
            # bit-trick seed + one Newton iteration on gpsimd (SBUF-only,
            # off every critical engine); broadcast over partitions with
            # a single K=1 outer product on PE.  Stage 1 (Newton) fires
            # one head later, stage 2 (broadcast+mul) at that head's last
            # block, so cross-engine latency never stalls anyone.
            MAGIC = 0x7EF311C3
            norm_s1, norm_s2 = [], []

            def norm_stage1(lsb):
                r0 = small.tile([1, W], F32, tag="nr0", bufs=1, name="nr0")
                nc.vector.tensor_scalar(
                    out=r0.bitcast(mybir.dt.uint32),
                    in0=lsb.bitcast(mybir.dt.uint32),
                    scalar1=0xFFFFFFFF, scalar2=None,
                    op0=mybir.AluOpType.bitwise_xor)
                # MAGIC - bits == (~bits) - (~MAGIC): avoids u32 wraparound
                nc.vector.tensor_scalar(
                    out=r0.bitcast(mybir.dt.uint32),
                    in0=r0.bitcast(mybir.dt.uint32),
                    scalar1=0xFFFFFFFF - MAGIC, scalar2=None,
                    op0=mybir.AluOpType.subtract)
                t = small.tile([1, W], F32, tag="nt", bufs=1, name="nt")
                nc.gpsimd.tensor_mul(out=t, in0=lsb, in1=r0)
                nc.gpsimd.tensor_scalar(
                    out=t, in0=t, scalar1=-1.0, scalar2=2.0,
                    op0=mybir.AluOpType.mult, op1=mybir.AluOpType.add)
                rcp = small.tile([1, W], F32, tag="nrcp", name="nrcp")
                nc.gpsimd.tensor_mul(out=rcp, in0=t, in1=r0)
                return rcp

            def norm_stage2(ysb, rcp, h, gsl):
                rb = prb.tile([P, W], F32, tag="rb", bufs=1, name="rb")
                nc.tensor.matmul(rb, onesr_s, rcp, start=True, stop=True)
                nc.vector.tensor_mul(out=yT[:, h, gsl], in0=ysb, in1=rb)

            for g in range(G):
                gsl = slice(g * W, (g + 1) * W)
                nblk = 4 * (g + 1)   # causal: tk blocks 0..4(g+1)-1
                cwork = make_c_chunks(g - 1, last=False) if g > 0 else []
                stride = max(1, (2 * nblk) // len(cwork)) if cwork else 0
                ci = 0
                bc = 0
                for h in range(NQ):
                    yps = pacc.tile([P, W], F32, tag="acc", bufs=2,
                                    name="yps")
                    # two exp-sum accumulators: acc_a owned by DVE
                    # (j%4 != 3), acc_b owned by gpsimd (j%4 == 3) -- no
                    # cross-engine read-modify-write on either tile.
                    acc_a = accp.tile([P, W], MM, tag="acca", name="acca")
                    acc_b = accp.tile([P, W], MM, tag="accb", name="accb")
                    if nblk < 5:
                        # g=0: acc_b's first block (j=1) covers cols 128:512
                        nc.gpsimd.memset(acc_b[:, 0:P], 0.0)
                    first_b = True
                    pend = [emit_S(g, h, 0, gsl)]
                    if nblk > 1:
                        pend.append(emit_S(g, h, 1, gsl))
                    for j in range(nblk):
                        if j + 2 < nblk:
                            pend.append(emit_S(g, h, j + 2, gsl))
                        sps, vsl = pend[j]
                        pt = ptp.tile([P, W], MM, tag="pt", bufs=6, name="pt")
                        nc.scalar.activation(out=pt[:, vsl], in_=sps[:, vsl],
                                             func=EXP)
                        jj = j - g * 4
                        if jj >= 0:
                            dsl = slice(jj * P, (jj + 1) * P)
                            nc.vector.tensor_mul(out=pt[:, dsl],
                                                 in0=pt[:, dsl], in1=mask_s)
                        nc.tensor.matmul(yps[:, vsl], Vt[:, j, :], pt[:, vsl],
                                         start=(j == 0), stop=(j == nblk - 1))
                        if j % 4 == 1:
                            # gps owns j%4==1 so the head's LAST add (j%4==3)
                            # is on the faster DVE and never gates the L-mm
                            if first_b:
                                # full-width copies are slow on gpsimd
                                nc.vector.tensor_copy(out=acc_b[:, vsl],
                                                      in_=pt[:, vsl])
                                first_b = False
                            else:
                                nc.gpsimd.tensor_add(out=acc_b[:, vsl],
                                                     in0=acc_b[:, vsl],
                                                     in1=pt[:, vsl])
                        elif j == 0:
                            nc.vector.tensor_copy(out=acc_a, in_=pt)
                        else:
                            nc.vector.tensor_add(out=acc_a[:, vsl],
                                                 in0=acc_a[:, vsl],
                                                 in1=pt[:, vsl])
                        bc += 1
                        if j == 1:
                            if norm_s2:
                                norm_s2.pop(0)()
                            if norm_s1:
                                norm_s1.pop()()
                        if h >= 2 and cwork and ci < len(cwork) \
                                and bc % stride == 0:
                            cwork[ci]()
                            ci += 1
                    # L[tq] = colsum(acc_a + acc_b) via 2-chain ones-matmul
                    lps = plps.tile([1, W], F32, tag="lps", bufs=2,
                                    name="lps")
                    nc.tensor.matmul(lps, ones_s, acc_a,
                                     start=True, stop=False)
                    nc.tensor.matmul(lps, ones_s, acc_b,
                                     start=False, stop=True)
                    lsb = small.tile([1, W], F32, tag="lsb", name="lsb")
                    nc.vector.tensor_copy(out=lsb, in_=lps)
                    # free the psum bank now; normalize later from SBUF
                    ysb = stage.tile([P, W], F32, tag="ysb", bufs=2, name="ysb")
                    nc.scalar.copy(out=ysb, in_=yps)

                    def s1(lsb=lsb, ysb=ysb, h=h, gsl=gsl):
                        rcp = norm_stage1(lsb)
                        norm_s2.append(
                            lambda: norm_stage2(ysb, rcp, h, gsl))
                    norm_s1.append(s1)
                while ci < len(cwork):
                    cwork[ci]()
                    ci += 1
            while norm_s1:
                norm_s1.pop()()
            while norm_s2:
                norm_s2.pop(0)()
            for ch in make_c_chunks(G - 1, last=True):
                ch()
            ctx_bc.close()

    nc.compile()
    return nc


def _phase_a_group(nc, tc, pa, stage, x_t, t, lhs_a, qT, kT, Vt, id_s,
                   rope_apply, a_hooks):
    tsl = slice(t * W, (t + 1) * W)
    for m in range(6):
        hook = a_hooks.pop((t, m), None)
        if hook is not None:
            hook()
        ps = pa.tile([P, W], F32, tag="psA", bufs=3, name="ps")
        for c in range(CB):
            nc.tensor.matmul(ps, lhs_a(m, c), x_t[:, c, :],
                             start=(c == 0), stop=(c == CB - 1))
        if m < 4:
            praw = stage.tile([P, W], F32, tag="raw", bufs=4, name="praw")
            nc.scalar.copy(out=praw, in_=ps)
            rope_apply(qT[:, m, tsl], praw, tsl)
        elif m == 4:
            praw = stage.tile([P, W], F32, tag="raw", bufs=4, name="praw")
            nc.scalar.copy(out=praw, in_=ps)
            rope_apply(kT[:, tsl], praw, tsl)
        else:
            vraw = stage.tile([P, W], MM, tag="vraw", name="vraw")
            nc.vector.tensor_copy(out=vraw, in_=ps)
            for jj in range(4):
                j = t * 4 + jj
                pvt = pa.tile([P, P], MM, tag="pvt", bufs=2, name="pvt")
                nc.tensor.transpose(pvt, vraw[:, jj * P:(jj + 1) * P], id_s)
                nc.vector.tensor_copy(out=Vt[:, j, :], in_=pvt)


def make_tables():
    inv = (10000.0 ** (-(np.arange(64, dtype=np.float32) / np.float32(64.0)))
           ).astype(np.float32)
    freqs = np.arange(T, dtype=np.float32)[:, None] * inv[None, :]   # [T, 64]
    cos64 = np.cos(freqs).T.astype(np.float32)                       # [64, T]
    sin64 = np.sin(freqs).T.astype(np.float32)
    cosT = np.concatenate([cos64, cos64], axis=0)                    # [128, T]
    msinT = np.concatenate([-sin64, sin64], axis=0)
    # [P, P] 0/1 mask: mask[tk, tq] = 1 where tk <= tq (causal allowed)
    mask = np.where(
        np.arange(P)[:, None] <= np.arange(P)[None, :],
        np.float32(1.0), np.float32(0.0))
    ident = np.eye(P, dtype=np.float32)
    return cosT, msinT, mask, ident


def _rearr(a, p=P):
    """[R, M] with R = n*p -> [p, n*M] so each partition line is
    contiguous in DRAM: out[pp, n*M + m] = a[n*p + pp, m]."""
    R, M = a.shape
    n = R // p
    return np.ascontiguousarray(
        a.reshape(n, p, M).transpose(1, 0, 2).reshape(p, n * M))


def _rearr_x(xT):
    """xT [C, T] -> [P, G*CB*W], t-group major: out[p, ((t*CB)+cb)*W + w]
    = xT[cb*P + p, t*W + w], so each (p, t) line is CB*W contiguous."""
    a = xT.reshape(CB, P, G, W).transpose(1, 2, 0, 3)
    return np.ascontiguousarray(a.reshape(P, G * CB * W))


def shard_inputs(x, Wq, Wk, Wv, Wproj):
    import ml_dtypes
    bf16 = ml_dtypes.bfloat16
    cosT, msinT, mask, ident = make_tables()
    scale = np.float32(1.0 / np.sqrt(np.float32(HD)))
    xRb = [_rearr_x(np.ascontiguousarray(x[b].T)).astype(bf16)
           for b in range(B)]
    in_maps = []
    for core in range(8):
        b, g = core // 4, core % 4
        in_maps.append({
            "xR": xRb[b],
            "wqr": _rearr(Wq[:, g * NQ * HD:(g + 1) * NQ * HD] * scale
                          ).astype(bf16),
            "wkr": _rearr(Wk[:, g * HD:(g + 1) * HD]).astype(bf16),
            "wvr": _rearr(Wv[:, g * HD:(g + 1) * HD]).astype(bf16),
            "wpr": _rearr(Wproj[g * NQ * HD:(g + 1) * NQ * HD, :]).astype(bf16),
            "cosT": cosT.astype(bf16), "msinT": msinT.astype(bf16),
            "mask01": mask.astype(bf16),
            "ident": ident.astype(bf16),
            "onescol": np.ones((P, 1), dtype=bf16),
            "onesrow": np.ones((1, P), dtype=np.float32),
        })
    return in_maps


_NC_CACHE = {}


def _get_nc():
    key = USE_F32R
    if key not in _NC_CACHE:
        _NC_CACHE[key] = build_nc()
    return _NC_CACHE[key]


def kernel(x, Wq, Wk, Wv, Wproj):
    from concourse.bass_utils import run_bass_kernel_spmd
    x = np.asarray(x, dtype=np.float32)
    Wq = np.asarray(Wq, dtype=np.float32)
    Wk = np.asarray(Wk, dtype=np.float32)
    Wv = np.asarray(Wv, dtype=np.float32)
    Wproj = np.asarray(Wproj, dtype=np.float32)
    nc = _get_nc()
    in_maps = shard_inputs(x, Wq, Wk, Wv, Wproj)
    res = run_bass_kernel_spmd(nc, in_maps, core_ids=list(range(8)))
    out = np.zeros((B, T, C), dtype=np.float32)
    for core in range(8):
        b = core // 4
        out[b] += np.asarray(res.results[core]["out"], dtype=np.float32)
    return out


# revision 47
# speedup vs baseline: 1.0153x; 1.0056x over previous
"""Causal self-attention (GQA + RoPE) Trainium2 Bass kernel, 8-core SPMD.

Problem shapes (hardcoded): B=2, T=2048, C=2048, NH=16, NKV=4, HD=128.

Sharding: 8 cores = (batch b in {0,1}) x (kv-group g in {0..3}).
Core c = b*4+g handles batch b, q-heads 4g..4g+3, kv-head g.
  - Wq column-parallel (512 cols/core), Wk/Wv column-parallel (128 cols/core),
    Wproj row-parallel (512 rows/core) -> per-core partial [T, C] outputs,
    host sums the 4 partials per batch.

Per-core dataflow ("transposed flash"):
  - Inputs pre-rearranged on host so every weight/x load is one big DMA
    with contiguous per-partition lines; all loads issued at t=0 spread
    over the sync/scalar/vector/gpsimd queues.
  - Projections computed in transposed layout: qT/kT [HD, T]
    (lhsT = W chunk, rhs = xT chunk), V transposed to [T, HD] via PE.
  - RoPE on qT/kT via partition-shifted SBUF copies + cos/sin tables.
  - Scores computed transposed: S^T[tk, tq] = matmul(lhsT=kT block, rhs=qT).
    No mask preload: exp runs on raw scores and the causal triangle of the
    diagonal 128-chunk is zeroed afterwards by a [128,128] vector multiply.
  - exp without max subtraction (scores are O(5) here; safe in fp32).
  - Row sums L[tq]: exp tiles are accumulated in bf16 SBUF tiles by the
    vector engine (even tk blocks) and gpsimd/pool engine (odd blocks);
    one 2-chain ones-matmul per head reduces the partition dim on PE.
  - 1/L: DMA-reshape L [1,512] -> [128,4] so the reciprocal runs on 128
    lanes (~60ns instead of 3.3us), then DRAM-bounce broadcast to
    [128,512]; normalization on DVE.
  - Phase B software-pipelined: S for block j+2 issued before PV_j so the
    PE never waits on the scalar-engine exp.
  - Wproj row-parallel partials in bf16; out-proj chunks for tq group g-1
    woven between attention blocks of group g to fill PE slack while the
    scalar engine runs exp; psum->sbuf copies alternate vector/gpsimd.
"""

import numpy as np

import concourse.bass as bass
import concourse.bacc as bacc
import concourse.mybir as mybir
import concourse.tile as tile

B, T, C = 2, 2048, 2048
NH, NKV, HD = 16, 4, 128
P = 128
W = 512            # wide tile (PSUM bank = 512 fp32)
TB = T // P        # 16 t blocks
CB = C // P        # 16 c chunks
G = T // W         # 4 tq groups
NQ = 4             # q heads per core

F32 = mybir.dt.float32

USE_F32R = False
MM = mybir.dt.bfloat16    # matmul-input compute dtype


def build_nc():
    nc = bacc.Bacc("TRN2", target_bir_lowering=False)
    xR = nc.dram_tensor("xR", (P, G * CB * W), MM, kind="ExternalInput")[:]
    wqr = nc.dram_tensor("wqr", (P, CB * NQ * HD), MM, kind="ExternalInput")[:]
    wkr = nc.dram_tensor("wkr", (P, CB * HD), MM, kind="ExternalInput")[:]
    wvr = nc.dram_tensor("wvr", (P, CB * HD), MM, kind="ExternalInput")[:]
    wpr = nc.dram_tensor("wpr", (P, NQ * C), MM, kind="ExternalInput")[:]
    cosT = nc.dram_tensor("cosT", (P, T), MM, kind="ExternalInput")[:]
    msinT = nc.dram_tensor("msinT", (P, T), MM, kind="ExternalInput")[:]
    mask01 = nc.dram_tensor("mask01", (P, P), MM, kind="ExternalInput")[:]
    ident = nc.dram_tensor("ident", (P, P), MM, kind="ExternalInput")[:]
    onescol = nc.dram_tensor("onescol", (P, 1), MM, kind="ExternalInput")[:]
    onesrow = nc.dram_tensor("onesrow", (1, P), F32, kind="ExternalInput")[:]
    out = nc.dram_tensor("out", (T, C), MM, kind="ExternalOutput")[:]

    EXP = mybir.ActivationFunctionType.Exp

    xr4 = xR.rearrange("p (t cb w) -> p t cb w", t=G, cb=CB)
    wq3 = wqr.rearrange("p (cb m) -> p cb m", cb=CB)
    wk3 = wkr.rearrange("p (cb m) -> p cb m", cb=CB)
    wv3 = wvr.rearrange("p (cb m) -> p cb m", cb=CB)
    wp3 = wpr.rearrange("p (hb c) -> p hb c", hb=NQ)

    with tile.TileContext(nc) as tc:
        with (
            tc.tile_pool(name="singles", bufs=1) as singles,
            tc.tile_pool(name="stage", bufs=3) as stage,
            tc.tile_pool(name="ptp", bufs=4) as ptp,
            tc.tile_pool(name="accp", bufs=2) as accp,
            tc.tile_pool(name="outp", bufs=2) as outp,
            tc.tile_pool(name="small", bufs=2) as small,
            tc.tile_pool(name="dramp", bufs=4, space="DRAM") as dramp,
        ):
            # ---- resident tiles ----
            qT = singles.tile([P, NQ, T], MM)       # roped q
            yT = singles.tile([P, NQ, T], MM)       # attention out (pre-proj)
            kT = singles.tile([P, T], MM)           # roped k, [hd, t]
            Vt = singles.tile([P, TB, HD], MM)      # [t_in_blk, blk, hd]
            cos_s = singles.tile([P, T], MM)
            msin_s = singles.tile([P, T], MM)
            mask_s = singles.tile([P, P], MM)       # 0/1 causal triangle
            id_s = singles.tile([P, P], MM)
            ones_s = singles.tile([P, 1], MM)       # column of ones
            onesr_s = singles.tile([1, P], F32)     # row of ones (outer lhsT)
            wqp = [singles.tile([P, 4, NQ * HD], MM, name=f"wqp{i}")
                   for i in range(4)]
            wkall = singles.tile([P, CB, HD], MM)
            wvall = singles.tile([P, CB, HD], MM)
            wpall = singles.tile([P, NQ, C], MM)

            # ---- all loads issued up front, spread over 4 queues ----
            # sync: weights + tables (first eighth of wq tiny so the very
            # first matmul chain can start ~7us in).
            nc.sync.dma_start(out=wqp[0][:, 0:1, :], in_=wq3[:, 0:1, :])
            nc.sync.dma_start(out=wqp[0][:, 1:2, :], in_=wq3[:, 1:2, :])
            nc.sync.dma_start(out=wqp[0][:, 2:4, :], in_=wq3[:, 2:4, :])
            nc.sync.dma_start(out=wqp[1], in_=wq3[:, 4:8, :])
            nc.sync.dma_start(out=wqp[2], in_=wq3[:, 8:12, :])
            nc.sync.dma_start(out=wqp[3], in_=wq3[:, 12:16, :])
            nc.sync.dma_start(out=cos_s, in_=cosT)
            nc.sync.dma_start(out=msin_s, in_=msinT)
            nc.sync.dma_start(out=wkall, in_=wk3)
            nc.sync.dma_start(out=wvall, in_=wv3)
            nc.sync.dma_start(out=mask_s, in_=mask01)
            nc.sync.dma_start(out=id_s, in_=ident)
            nc.sync.dma_start(out=ones_s, in_=onescol)
            nc.sync.dma_start(out=onesr_s, in_=onesrow)

            def rope_apply(dst, praw, tsl):
                # dst[d,:] = praw[d,:]*cos[d,:] + rot(praw)[d,:]*msin[d,:]
                # rot swaps halves; the rotate-half sign is folded into msin.
                tmp = stage.tile([P, W], F32, tag="ropetmp", bufs=2)
                nc.gpsimd.dma_start(out=tmp[0:64, :], in_=praw[64:128, :])
                nc.gpsimd.dma_start(out=tmp[64:128, :], in_=praw[0:64, :])
                nc.vector.tensor_mul(out=dst, in0=praw, in1=cos_s[:, tsl])
                nc.vector.tensor_mul(out=tmp, in0=tmp, in1=msin_s[:, tsl])
                nc.vector.tensor_add(out=dst, in0=dst, in1=tmp)

            # ---- phase A: QKV projections + rope + V transpose ----
            def lhs_a(m, c):
                if m < 4:
                    return wqp[c // 4][:, c % 4, m * P:(m + 1) * P]
                w = wkall if m == 4 else wvall
                return w[:, c, 0:P]

            with (
                tc.tile_pool(name="pa", bufs=1, space="PSUM") as pa,
                tc.tile_pool(name="xin", bufs=2) as xin,
            ):
                xts = [xin.tile([P, CB, W], MM, tag="x", bufs=4,
                                name=f"x_t{t}") for t in range(G)]
                # x group 0 per-chunk so the first chain streams at DMA
                # arrival rate instead of waiting on monolithic transfers.
                for cb in range(CB):
                    nc.scalar.dma_start(out=xts[0][:, cb:cb + 1, :],
                                        in_=xr4[:, 0, cb:cb + 1, :])
                # later x groups + wp issued mid-phase-A so the early
                # DMA bandwidth all goes to wq/x0
                a_hooks = {
                    (0, 1): lambda: nc.gpsimd.dma_start(
                        out=xts[1], in_=xr4[:, 1, :, :]),
                    (0, 2): lambda: nc.scalar.dma_start(
                        out=xts[2], in_=xr4[:, 2, :, :]),
                    (0, 4): lambda: nc.gpsimd.dma_start(
                        out=xts[3], in_=xr4[:, 3, :, :]),
                    (1, 0): lambda: nc.scalar.dma_start(
                        out=wpall, in_=wp3),
                }
                for t in range(G):
                    _phase_a_group(nc, tc, pa, stage, xts[t], t,
                                   lhs_a, qT, kT, Vt, id_s, rope_apply,
                                   a_hooks)

            # ---- phases B+C psum pools (A's pool released above) ----
            import contextlib
            ctx_bc = contextlib.ExitStack()
            pmm = ctx_bc.enter_context(
                tc.tile_pool(name="pmm", bufs=1, space="PSUM"))
            pacc = ctx_bc.enter_context(
                tc.tile_pool(name="pacc", bufs=1, space="PSUM"))
            plps = ctx_bc.enter_context(
                tc.tile_pool(name="plps", bufs=1, space="PSUM"))
            prb = ctx_bc.enter_context(
                tc.tile_pool(name="prb", bufs=1, space="PSUM"))

            def emit_S(g, h, j, gsl):
                """Score matmul for tk block j; returns (psum tile, colslice).
                No mask preload; the diagonal triangle is zeroed post-exp."""
                jj = j - g * 4
                vst = max(jj, 0) * P
                vsl = slice(vst, W)
                sps = pmm.tile([P, W], F32, tag="mm", bufs=3, name="sps")
                nc.tensor.matmul(sps[:, vsl],
                                 kT[:, j * P:(j + 1) * P],
                                 qT[:, h, g * W + vst:(g + 1) * W],
                                 start=True, stop=True)
                return sps, vsl

            ost_state = {}

            def make_c_chunks(gg, last):
                """Output projection for tq group gg as 16 (i, cc) closures."""
                work = []
                for i in range(4 * gg, 4 * gg + 4):
                    for cc in range(4):
                        def chunk(i=i, cc=cc):
                            if cc == 0:
                                ost_state[0] = outp.tile([P, C], MM,
                                                         tag="ost", name="ost")
                            ost = ost_state[0]
                            csl = slice(cc * W, (cc + 1) * W)
                            ops = pmm.tile([P, W], F32, tag="mm", bufs=3,
                                           name="ops")
                            for hb in range(NQ):
                                nc.tensor.matmul(
                                    ops,
                                    yT[:, hb, i * P:(i + 1) * P],
                                    wpall[:, hb, csl],
                                    start=(hb == 0), stop=(hb == NQ - 1))
                            if cc % 4 == 3:
                                nc.vector.tensor_copy(out=ost[:, csl],
                                                      in_=ops)
                            else:
                                nc.scalar.copy(out=ost[:, csl], in_=ops)
                            if last:
                                # final group: store per-cc to shrink tail
                                q = (nc.sync, nc.gpsimd, nc.sync,
                                     nc.scalar)[cc]
                                q.dma_start(out=out[i * P:(i + 1) * P, csl],
                                            in_=ost[:, csl])
                            elif cc == 3:
                                q = nc.sync if i % 2 == 0 else nc.gpsimd
                                q.dma_start(out=out[i * P:(i + 1) * P, :],
                                            in_=ost)
                        work.append(chunk)
                return work

            # ---- phase B: attention, with phase-C chunks woven in ----
            # normalization: L via two N=512 ones-matmuls; 1/L via a
            # bit-trick seed + one Newton iteration on gpsimd (SBUF-only,
            # off every critical engine); broadcast over partitions with
            # a single K=1 outer product on PE.  Stage 1 (Newton) fires
            # one head later, stage 2 (broadcast+mul) at that head's last
            # block, so cross-engine latency never stalls anyone.
            MAGIC = 0x7EF311C3
            norm_s1, norm_s2 = [], []

            def norm_stage1(lsb):
                r0 = small.tile([1, W], F32, tag="nr0", bufs=1, name="nr0")
                nc.vector.tensor_scalar(
                    out=r0.bitcast(mybir.dt.uint32),
                    in0=lsb.bitcast(mybir.dt.uint32),
                    scalar1=0xFFFFFFFF, scalar2=None,
                    op0=mybir.AluOpType.bitwise_xor)
                # MAGIC - bits == (~bits) - (~MAGIC): avoids u32 wraparound
                nc.vector.tensor_scalar(
                    out=r0.bitcast(mybir.dt.uint32),
                    in0=r0.bitcast(mybir.dt.uint32),
                    scalar1=0xFFFFFFFF - MAGIC, scalar2=None,
                    op0=mybir.AluOpType.subtract)
                t = small.tile([1, W], F32, tag="nt", bufs=1, name="nt")
                nc.gpsimd.tensor_mul(out=t, in0=lsb, in1=r0)
                nc.gpsimd.tensor_scalar(
                    out=t, in0=t, scalar1=-1.0, scalar2=2.0,
                    op0=mybir.AluOpType.mult, op1=mybir.AluOpType.add)
                rcp = small.tile([1, W], F32, tag="nrcp", name="nrcp")
                nc.gpsimd.tensor_mul(out=rcp, in0=t, in1=r0)
                return rcp

            def norm_stage2(ysb, rcp, h, gsl):
                rb = prb.tile([P, W], F32, tag="rb", bufs=1, name="rb")
                nc.tensor.matmul(rb, onesr_s, rcp, start=True, stop=True)
                nc.vector.tensor_mul(out=yT[:, h, gsl], in0=ysb, in1=rb)

            for g in range(G):
                gsl = slice(g * W, (g + 1) * W)
                nblk = 4 * (g + 1)   # causal: tk blocks 0..4(g+1)-1
                cwork = make_c_chunks(g - 1, last=False) if g > 0 else []
                stride = max(1, (2 * nblk) // len(cwork)) if cwork else 0
                ci = 0
                bc = 0
                for h in range(NQ):
                    yps = pacc.tile([P, W], F32, tag="acc", bufs=2,
                                    name="yps")
                    # two exp-sum accumulators: acc_a owned by DVE
                    # (j%4 != 3), acc_b owned by gpsimd (j%4 == 3) -- no
                    # cross-engine read-modify-write on either tile.
                    acc_a = accp.tile([P, W], MM, tag="acca", name="acca")
                    acc_b = accp.tile([P, W], MM, tag="accb", name="accb")
                    if nblk < 5:
                        # g=0: acc_b is unused (its only candidate block
                        # j=3 is the final block, routed to DVE/acc_a)
                        nc.gpsimd.memset(acc_b, 0.0)
                    first_b = True
                    pend = [emit_S(g, h, 0, gsl)]
                    if nblk > 1:
                        pend.append(emit_S(g, h, 1, gsl))
                    for j in range(nblk):
                        if j + 2 < nblk:
                            pend.append(emit_S(g, h, j + 2, gsl))
                        sps, vsl = pend[j]
                        pt = ptp.tile([P, W], MM, tag="pt", bufs=6, name="pt")
                        nc.scalar.activation(out=pt[:, vsl], in_=sps[:, vsl],
                                             func=EXP)
                        jj = j - g * 4
                        if jj >= 0:
                            dsl = slice(jj * P, (jj + 1) * P)
                            nc.vector.tensor_mul(out=pt[:, dsl],
                                                 in0=pt[:, dsl], in1=mask_s)
                        nc.tensor.matmul(yps[:, vsl], Vt[:, j, :], pt[:, vsl],
                                         start=(j == 0), stop=(j == nblk - 1))
                        if j % 4 == 3 and j != nblk - 1:
                            # the final block always goes to DVE/acc_a so
                            # the slow pool engine never gates the L-matmul
                            if first_b:
                                # full-width copies are slow on gpsimd
                                nc.vector.tensor_copy(out=acc_b[:, vsl],
                                                      in_=pt[:, vsl])
                                first_b = False
                            else:
                                nc.gpsimd.tensor_add(out=acc_b[:, vsl],
                                                     in0=acc_b[:, vsl],
                                                     in1=pt[:, vsl])
                        elif j == 0:
                            nc.vector.tensor_copy(out=acc_a, in_=pt)
                        else:
                            nc.vector.tensor_add(out=acc_a[:, vsl],
                                                 in0=acc_a[:, vsl],
                                                 in1=pt[:, vsl])
                        bc += 1
                        if j == 1:
                            if norm_s2:
                                norm_s2.pop(0)()
                            if norm_s1:
                                norm_s1.pop()()
                        if h >= 2 and cwork and ci < len(cwork) \
                                and bc % stride == 0:
                            cwork[ci]()
                            ci += 1
                    # L[tq] = colsum(acc_a + acc_b) via 2-chain ones-matmul
                    lps = plps.tile([1, W], F32, tag="lps", bufs=2,
                                    name="lps")
                    nc.tensor.matmul(lps, ones_s, acc_a,
                                     start=True, stop=False)
                    nc.tensor.matmul(lps, ones_s, acc_b,
                                     start=False, stop=True)
                    lsb = small.tile([1, W], F32, tag="lsb", name="lsb")
                    nc.vector.tensor_copy(out=lsb, in_=lps)
                    # free the psum bank now; normalize later from SBUF
                    ysb = stage.tile([P, W], F32, tag="ysb", bufs=2, name="ysb")
                    nc.scalar.copy(out=ysb, in_=yps)

                    def s1(lsb=lsb, ysb=ysb, h=h, gsl=gsl):
                        rcp = norm_stage1(lsb)
                        norm_s2.append(
                            lambda: norm_stage2(ysb, rcp, h, gsl))
                    norm_s1.append(s1)
                while ci < len(cwork):
                    cwork[ci]()
                    ci += 1
            while norm_s1:
                norm_s1.pop()()
            while norm_s2:
                norm_s2.pop(0)()
            for ch in make_c_chunks(G - 1, last=True):
                ch()
            ctx_bc.close()

    nc.compile()
    return nc


def _phase_a_group(nc, tc, pa, stage, x_t, t, lhs_a, qT, kT, Vt, id_s,
                   rope_apply, a_hooks):
    tsl = slice(t * W, (t + 1) * W)
    for m in range(6):
        hook = a_hooks.pop((t, m), None)
        if hook is not None:
            hook()
        ps = pa.tile([P, W], F32, tag="psA", bufs=3, name="ps")
        for c in range(CB):
            nc.tensor.matmul(ps, lhs_a(m, c), x_t[:, c, :],
                             start=(c == 0), stop=(c == CB - 1))
        if m < 4:
            praw = stage.tile([P, W], F32, tag="raw", bufs=4, name="praw")
            nc.scalar.copy(out=praw, in_=ps)
            rope_apply(qT[:, m, tsl], praw, tsl)
        elif m == 4:
            praw = stage.tile([P, W], F32, tag="raw", bufs=4, name="praw")
            nc.scalar.copy(out=praw, in_=ps)
            rope_apply(kT[:, tsl], praw, tsl)
        else:
            vraw = stage.tile([P, W], MM, tag="vraw", name="vraw")
            nc.vector.tensor_copy(out=vraw, in_=ps)
            for jj in range(4):
                j = t * 4 + jj
                pvt = pa.tile([P, P], MM, tag="pvt", bufs=2, name="pvt")
                nc.tensor.transpose(pvt, vraw[:, jj * P:(jj + 1) * P], id_s)
                nc.vector.tensor_copy(out=Vt[:, j, :], in_=pvt)


def make_tables():
    inv = (10000.0 ** (-(np.arange(64, dtype=np.float32) / np.float32(64.0)))
           ).astype(np.float32)
    freqs = np.arange(T, dtype=np.float32)[:, None] * inv[None, :]   # [T, 64]
    cos64 = np.cos(freqs).T.astype(np.float32)                       # [64, T]
    sin64 = np.sin(freqs).T.astype(np.float32)
    cosT = np.concatenate([cos64, cos64], axis=0)                    # [128, T]
    msinT = np.concatenate([-sin64, sin64], axis=0)
    # [P, P] 0/1 mask: mask[tk, tq] = 1 where tk <= tq (causal allowed)
    mask = np.where(
        np.arange(P)[:, None] <= np.arange(P)[None, :],
        np.float32(1.0), np.float32(0.0))
    ident = np.eye(P, dtype=np.float32)
    return cosT, msinT, mask, ident


def _rearr(a, p=P):
    """[R, M] with R = n*p -> [p, n*M] so each partition line is
    contiguous in DRAM: out[pp, n*M + m] = a[n*p + pp, m]."""
    R, M = a.shape
    n = R // p
    return np.ascontiguousarray(
        a.reshape(n, p, M).transpose(1, 0, 2).reshape(p, n * M))


def _rearr_x(xT):
    """xT [C, T] -> [P, G*CB*W], t-group major: out[p, ((t*CB)+cb)*W + w]
    = xT[cb*P + p, t*W + w], so each (p, t) line is CB*W contiguous."""
    a = xT.reshape(CB, P, G, W).transpose(1, 2, 0, 3)
    return np.ascontiguousarray(a.reshape(P, G * CB * W))


def shard_inputs(x, Wq, Wk, Wv, Wproj):
    import ml_dtypes
    bf16 = ml_dtypes.bfloat16
    cosT, msinT, mask, ident = make_tables()
    scale = np.float32(1.0 / np.sqrt(np.float32(HD)))
    xRb = [_rearr_x(np.ascontiguousarray(x[b].T)).astype(bf16)
           for b in range(B)]
    in_maps = []
    for core in range(8):
        b, g = core // 4, core % 4
        in_maps.append({
            "xR": xRb[b],
            "wqr": _rearr(Wq[:, g * NQ * HD:(g + 1) * NQ * HD] * scale
                          ).astype(bf16),
            "wkr": _rearr(Wk[:, g * HD:(g + 1) * HD]).astype(bf16),
            "wvr": _rearr(Wv[:, g * HD:(g + 1) * HD]).astype(bf16),
            "wpr": _rearr(Wproj[g * NQ * HD:(g + 1) * NQ * HD, :]).astype(bf16),
            "cosT": cosT.astype(bf16), "msinT": msinT.astype(bf16),
            "mask01": mask.astype(bf16),
            "ident": ident.astype(bf16),
            "onescol": np.ones((P, 1), dtype=bf16),
            "onesrow": np.ones((1, P), dtype=np.float32),
        })
    return in_maps


_NC_CACHE = {}


def _get_nc():
    key = USE_F32R
    if key not in _NC_CACHE:
        _NC_CACHE[key] = build_nc()
    return _NC_CACHE[key]


def kernel(x, Wq, Wk, Wv, Wproj):
    from concourse.bass_utils import run_bass_kernel_spmd
    x = np.asarray(x, dtype=np.float32)
    Wq = np.asarray(Wq, dtype=np.float32)
    Wk = np.asarray(Wk, dtype=np.float32)
    Wv = np.asarray(Wv, dtype=np.float32)
    Wproj = np.asarray(Wproj, dtype=np.float32)
    nc = _get_nc()
    in_maps = shard_inputs(x, Wq, Wk, Wv, Wproj)
    res = run_bass_kernel_spmd(nc, in_maps, core_ids=list(range(8)))
    out = np.zeros((B, T, C), dtype=np.float32)
    for core in range(8):
        b = core // 4
        out[b] += np.asarray(res.results[core]["out"], dtype=np.float32)
    return out
